# revision 18
# baseline (speedup 1.0000x reference)
"""Trainium2 Bass kernel for nn_CSM_62216896250023 (dense_cnn).

Computation: H = sigmoid-conv-chain(emb[X]) with four per-channel seq
convolutions (h=2,2,3,3) -> output (16384, 1024) fp32.

Strategy (8 NeuronCores, pure data parallel over the batch):
- Host: round the embedding table to bf16 once; per core, build int16
  gather-index tiles in the SWDGE wrap layout; pack per-channel conv
  weights as [128, 80] fp32 per-partition scalar columns.
- Device per core (batch shard of 2048, tiled by 256):
  * dma_gather(transpose=True) pulls 7*256 embedding rows directly in
    channel-major SBUF layout [128, 8, 1792] bf16 (partition = channel%128).
  * Each conv layer y = sigmoid(sum_t k_t * x_t) uses the ratio weights
    k_t/k_last: the tap chain runs as fused scalar_tensor_tensor ops
    ((x*k')+acc, bf16) and the final k_last multiply is folded into the
    ACT sigmoid's per-partition scale. Token order inside a tile is
    position-major, so one instruction covers all output positions of a
    layer.
  * Output is written channel-major ([128, t, j, b] fp32); the host
    permutes it back to (batch, channel) - a pure layout change.
"""
import numpy as np
import ml_dtypes

import concourse.bacc as bacc
import concourse.tile as tile
import concourse.bass_utils as bass_utils
from concourse import mybir

bf16 = ml_dtypes.bfloat16

VOCAB, EMBED, BATCH, SEQ = 32000, 1024, 16384, 7
NCORES = 8
BCORE = BATCH // NCORES          # 2048
BT = 256                         # batch rows per tile
NT = BCORE // BT                 # 8 tiles per core
NIDX = SEQ * BT                  # 1792 gathered rows per tile
SCOLS = NIDX // 16               # 112 idx columns per tile
JB = EMBED // 128                # 8 channel blocks

# weight column layout inside the packed [128, 80] fp32 tensor
COL = {"kd1_0": 0, "sc1": 8, "kd2_0": 16, "sc2": 24,
       "kd3_0": 32, "kd3_1": 40, "sc3": 48,
       "kd4_0": 56, "kd4_1": 64, "sc4": 72}
NWCOLS = 80

# channel blocks assigned to the tensor engine (accumulated diagonal
# matmuls into PSUM); the rest stay on DVE (fused stt chain).
PE_JS = (0, 3, 6)
_DIAG_NAMES = ["kd1_0", "kd2_0", "kd3_0", "kd3_1", "kd4_0", "kd4_1"]
NDIAG = len(PE_JS) * len(_DIAG_NAMES) + 1          # +1 shared identity

_prog_cache = {}


def _build_program():
    if "nc" in _prog_cache:
        return _prog_cache["nc"]
    f32, b16, i16 = mybir.dt.float32, mybir.dt.bfloat16, mybir.dt.int16
    SIG = mybir.ActivationFunctionType.Sigmoid
    MUL, ADD = mybir.AluOpType.mult, mybir.AluOpType.add

    nc = bacc.Bacc("TRN2", target_bir_lowering=False, debug=False)
    table = nc.dram_tensor("table", [VOCAB, EMBED], b16, kind="ExternalInput")
    idx = nc.dram_tensor("idx", [128, NT * SCOLS], i16, kind="ExternalInput")
    wts = nc.dram_tensor("wts", [128, NWCOLS], f32, kind="ExternalInput")
    diags = nc.dram_tensor("diags", [128, NDIAG * 128], b16,
                           kind="ExternalInput")
    # channel-major raw output: [128, t, j, b]
    out = nc.dram_tensor("out", [128, NT * JB * BT], f32,
                         kind="ExternalOutput")

    with tile.TileContext(nc) as tc:
        with tc.tile_pool(name="const", bufs=1) as cpool, \
             tc.tile_pool(name="xt", bufs=2) as xpool, \
             tc.tile_pool(name="s1", bufs=4) as s1p, \
             tc.tile_pool(name="y1", bufs=11) as y1p, \
             tc.tile_pool(name="s2", bufs=4) as s2p, \
             tc.tile_pool(name="y2", bufs=11) as y2p, \
             tc.tile_pool(name="t3", bufs=3) as t3p, \
             tc.tile_pool(name="y3", bufs=11) as y3p, \
             tc.tile_pool(name="t4", bufs=3) as t4p, \
             tc.tile_pool(name="y4", bufs=2) as y4p, \
             tc.tile_pool(name="ps", bufs=2, space="PSUM") as pspool:

            idx_sb = cpool.tile([128, NT * SCOLS], i16)
            nc.sync.dma_start(idx_sb[:], idx.ap())
            w_sb = cpool.tile([128, NWCOLS], f32)
            nc.sync.dma_start(w_sb[:], wts.ap())
            d_sb = cpool.tile([128, NDIAG * 128], b16)
            nc.sync.dma_start(d_sb[:], diags.ap())

            def wc(name, j):
                c = COL[name] + j
                return w_sb[:, c:c + 1]

            def dg(name, j):
                s = PE_JS.index(j) * len(_DIAG_NAMES) + _DIAG_NAMES.index(name)
                return d_sb[:, s * 128:(s + 1) * 128]

            id_ap = d_sb[:, (NDIAG - 1) * 128:NDIAG * 128]

            def pe_layer(ps, taps, fd):
                nchunks = (fd + 511) // 512
                for ti, (dap, srcs) in enumerate(taps):
                    for ci in range(nchunks):
                        c0, c1 = ci * 512, min((ci + 1) * 512, fd)
                        nc.tensor.matmul(
                            ps[:, c0:c1], dap, srcs[:, c0:c1],
                            start=(ti == 0), stop=(ti == len(taps) - 1))

            B = BT
            for t in range(NT):
                xt = xpool.tile([128, JB, NIDX], b16)
                nc.gpsimd.dma_gather(
                    xt[:], table.ap(), idx_sb[:, t * SCOLS:(t + 1) * SCOLS],
                    NIDX, NIDX, EMBED, transpose=True, single_packet=False)

                y4 = y4p.tile([128, JB, B], mybir.dt.float32)

                def do_pe_j(j):
                    xj = xt[:, j, :]
                    ps1 = pspool.tile([128, 6 * B], mybir.dt.float32, tag="ps")
                    pe_layer(ps1, [(dg("kd1_0", j), xj[:, 0:6 * B]),
                                   (id_ap, xj[:, B:7 * B])], 6 * B)
                    y1 = y1p.tile([128, 6 * B], b16, tag="y1")
                    nc.scalar.activation(y1[:], ps1[:], SIG, scale=wc("sc1", j))
                    ps2 = pspool.tile([128, 5 * B], mybir.dt.float32, tag="ps")
                    pe_layer(ps2, [(dg("kd2_0", j), y1[:, 0:5 * B]),
                                   (id_ap, y1[:, B:6 * B])], 5 * B)
                    y2 = y2p.tile([128, 5 * B], b16, tag="y2")
                    nc.scalar.activation(y2[:], ps2[:], SIG, scale=wc("sc2", j))
                    ps3 = pspool.tile([128, 3 * B], mybir.dt.float32, tag="ps")
                    pe_layer(ps3, [(dg("kd3_0", j), y2[:, 0:3 * B]),
                                   (dg("kd3_1", j), y2[:, B:4 * B]),
                                   (id_ap, y2[:, 2 * B:5 * B])], 3 * B)
                    y3 = y3p.tile([128, 3 * B], b16, tag="y3")
                    nc.scalar.activation(y3[:], ps3[:], SIG, scale=wc("sc3", j))
                    ps4 = pspool.tile([128, B], mybir.dt.float32, tag="ps")
                    pe_layer(ps4, [(dg("kd4_0", j), y3[:, 0:B]),
                                   (dg("kd4_1", j), y3[:, B:2 * B]),
                                   (id_ap, y3[:, 2 * B:3 * B])], B)
                    nc.scalar.activation(y4[:, j, :], ps4[:], SIG,
                                         scale=wc("sc4", j))

                def do_dve_j(j):
                    xj = xt[:, j, :]
                    s1 = s1p.tile([128, 6 * B], b16, tag="s1")
                    nc.vector.scalar_tensor_tensor(
                        s1[:], xj[:, 0:6 * B], wc("kd1_0", j), xj[:, B:7 * B],
                        MUL, ADD)
                    y1 = y1p.tile([128, 6 * B], b16, tag="y1")
                    nc.scalar.activation(y1[:], s1[:], SIG, scale=wc("sc1", j))
                    s2 = s2p.tile([128, 5 * B], b16, tag="s2")
                    nc.vector.scalar_tensor_tensor(
                        s2[:], y1[:, 0:5 * B], wc("kd2_0", j), y1[:, B:6 * B],
                        MUL, ADD)
                    y2 = y2p.tile([128, 5 * B], b16, tag="y2")
                    nc.scalar.activation(y2[:], s2[:], SIG, scale=wc("sc2", j))
                    ta = t3p.tile([128, 3 * B], b16, tag="t3a")
                    nc.vector.scalar_tensor_tensor(
                        ta[:], y2[:, B:4 * B], wc("kd3_1", j), y2[:, 2 * B:5 * B],
                        MUL, ADD)
                    tb = t3p.tile([128, 3 * B], b16, tag="t3b")
                    nc.vector.scalar_tensor_tensor(
                        tb[:], y2[:, 0:3 * B], wc("kd3_0", j), ta[:],
                        MUL, ADD)
                    y3 = y3p.tile([128, 3 * B], b16, tag="y3")
                    nc.scalar.activation(y3[:], tb[:], SIG, scale=wc("sc3", j))
                    tc4 = t4p.tile([128, B], b16, tag="t4a")
                    nc.vector.scalar_tensor_tensor(
                        tc4[:], y3[:, B:2 * B], wc("kd4_1", j), y3[:, 2 * B:3 * B],
                        MUL, ADD)
                    td = t4p.tile([128, B], b16, tag="t4b")
                    nc.vector.scalar_tensor_tensor(
                        td[:], y3[:, 0:B], wc("kd4_0", j), tc4[:],
                        MUL, ADD)
                    nc.scalar.activation(y4[:, j, :], td[:], SIG,
                                         scale=wc("sc4", j))

                for j in range(JB):
                    if j in PE_JS:
                        do_pe_j(j)
                    else:
                        do_dve_j(j)

                nc.sync.dma_start(
                    out.ap()[:, t * JB * B:(t + 1) * JB * B], y4[:])

    nc.compile()
    _prog_cache["nc"] = nc
    return nc


def _pack_weights(conv1, conv2, conv3, conv4):
    w = np.zeros((128, NWCOLS), np.float32)

    def put(name, arr):
        m = np.asarray(arr, np.float32).reshape(JB, 128).T  # [128, 8]
        w[:, COL[name]:COL[name] + JB] = m

    for L, k in ((1, conv1), (2, conv2), (3, conv3), (4, conv4)):
        k = np.asarray(k, np.float32)
        h = k.shape[0]
        for tpos in range(h - 1):
            put(f"kd{L}_{tpos}", k[tpos] / k[h - 1])
        put(f"sc{L}", k[h - 1])
    return w


def _pack_diags(conv1, conv2, conv3, conv4):
    ks = {}
    for L, k in ((1, conv1), (2, conv2), (3, conv3), (4, conv4)):
        k = np.asarray(k, np.float32)
        h = k.shape[0]
        for tpos in range(h - 1):
            ks[f"kd{L}_{tpos}"] = (k[tpos] / k[h - 1]).reshape(JB, 128)
    d = np.zeros((128, NDIAG * 128), np.float32)
    for ji, j in enumerate(PE_JS):
        for ni, name in enumerate(_DIAG_NAMES):
            s = ji * len(_DIAG_NAMES) + ni
            np.fill_diagonal(d[:, s * 128:(s + 1) * 128], ks[name][j])
    np.fill_diagonal(d[:, (NDIAG - 1) * 128:NDIAG * 128], 1.0)
    return d.astype(bf16)


def _make_idx(Xc):
    """Xc: (BCORE, SEQ) int array -> [128, NT*SCOLS] int16 in SWDGE wrap
    layout, position-major token order per tile."""
    out = np.zeros((128, NT * SCOLS), np.int16)
    for t in range(NT):
        flat = Xc[t * BT:(t + 1) * BT, :].T.reshape(-1)  # [SEQ*BT] pos-major
        wrap = flat.reshape(SCOLS, 16).T.astype(np.int16)  # [16, SCOLS]
        for m in range(8):
            out[16 * m:16 * m + 16, t * SCOLS:(t + 1) * SCOLS] = wrap
    return out


def _unpermute(raw):
    """raw: [128, NT*JB*BT] f32 channel-major -> (BCORE, EMBED)."""
    a = raw.reshape(128, NT, JB, BT)
    # out[b, c] with b = t*BT + bl, c = j*128 + p  <-  a[p, t, j, bl]
    return np.ascontiguousarray(
        a.transpose(1, 3, 2, 0).reshape(BCORE, EMBED))


def run(X, emb, conv1, conv2, conv3, conv4, **spmd_kwargs):
    X = np.asarray(X)
    emb = np.asarray(emb, np.float32)
    nc = _build_program()

    table = emb.astype(bf16)
    wpack = _pack_weights(conv1, conv2, conv3, conv4)
    dpack = _pack_diags(conv1, conv2, conv3, conv4)

    in_maps = []
    for c in range(NCORES):
        Xc = X[c * BCORE:(c + 1) * BCORE]
        in_maps.append({"table": table, "idx": _make_idx(Xc), "wts": wpack,
                        "diags": dpack})

    res = bass_utils.run_bass_kernel_spmd(nc, in_maps,
                                          core_ids=list(range(NCORES)),
                                          **spmd_kwargs)
    out = np.concatenate(
        [_unpermute(res.results[c]["out"]) for c in range(NCORES)], axis=0)
    return out, res


def kernel(X, emb, conv1, conv2, conv3, conv4):
    out, _ = run(X, emb, conv1, conv2, conv3, conv4)
    return out
